# revision 6
# baseline (speedup 1.0000x reference)
"""Centroid-similarity (ProtoNet softmax) kernel for 8 trn2 NeuronCores.

Math (per reference):
    counts   = sum_n y[n, c]
    cent     = (y^T @ E) / max(counts, 1)          # divide_no_nan
    out      = softmax(-(|e|^2 + |c|^2 - 2 e.c), axis=C)
softmax is invariant to per-row constants, so |e|^2 drops out:
    out      = softmax(2*cross - sq_c), cross = E @ cent^T, sq_c = |cent|^2

Distribution: data-parallel over N. Each core gets an N/8 = 2048-row shard,
computes partial (y^T E | counts) stats, AllReduces the [C, D+1] stats, then
computes its own 2048 x C block of logits + softmax.

v2 design (vs the fp32r baseline):
- All matmuls run in fp16 (11 mantissa bits, same precision class as the
  fp32r path but 2-4x the PE throughput). mm1 is error-compensated: E is
  split into fp16 hi + fp16 residual lo, both accumulated into the same
  PSUM, so global centroids are fp32-accurate. mm2 keeps single fp16
  (simulated end-to-end rel err ~1.3e-2 vs the 2e-2 gate).
- E^T (needed because mm2 contracts over D) is built by the DMA transpose
  XBAR instead of 128 PE transposes: fp16 E goes to a DRAM scratch after
  the load finishes (so the writes don't steal load bandwidth), then two
  large dma_start_transpose ops land E^T in SBUF. This frees ~48us of PE.
- A tiny warm-up AllReduce is triggered at t=0 from the (otherwise idle)
  gpsimd queue so the one-time CC barrier/setup (~35us) overlaps the input
  load instead of serializing in front of the real stats AllReduce.
- The AllReduce-adjacent DMAs trigger from the ACT queue (also hwdge) so
  they are not stuck behind DMA triggers on the sync queue.
"""

import numpy as np

import concourse.bacc as bacc
import concourse.bass as bass
import concourse.mybir as mybir
import concourse.tile as tile
from concourse import masks
from concourse.bass_utils import run_bass_kernel_spmd
from concourse.tile import add_dep_helper

N, C, D = 16384, 128, 1024
CORES = 8
NS = N // CORES          # 2048 rows per core
P = 128                  # partition dim
NCH = NS // P            # 16 n-chunks per core
DCH = D // P             # 8 d-chunks
NB = NS // 512           # 4 moving-dim blocks for matmul #2

F32 = mybir.dt.float32
F16 = mybir.dt.float16
BF16 = mybir.dt.bfloat16

AF = mybir.ActivationFunctionType
AX = mybir.AxisListType


def _build_kernel(tc: tile.TileContext, emb: bass.AP, yt: bass.AP, out: bass.AP):
    nc = tc.nc

    with (
        tc.tile_pool(name="const", bufs=1) as const_pool,
        tc.tile_pool(name="persist", bufs=1) as persist,
        tc.tile_pool(name="echunks", bufs=4) as e_pool,
        tc.tile_pool(name="elchunks", bufs=4) as el_pool,
        tc.tile_pool(name="ychunks", bufs=4) as y_pool,
        tc.tile_pool(name="dram", bufs=1, space="DRAM") as dram_pool,
        tc.tile_pool(name="smalls", bufs=1) as smalls,
    ):
        # ---- warm-up collective: absorb the one-time CC barrier/setup ----
        # Triggered from gpsimd at t~0; the 30us+ cross-core rendezvous then
        # overlaps the input load instead of the real stats AllReduce.
        w_sb = const_pool.tile([P, 1], F32)
        nc.gpsimd.memset(w_sb[:], 0.0)
        w_in = dram_pool.tile([P, 1], F32)
        w_out = dram_pool.tile([P, 1], F32)
        nc.gpsimd.dma_start(out=w_in[:], in_=w_sb[:])
        nc.gpsimd.collective_compute(
            "AllReduce",
            mybir.AluOpType.add,
            replica_groups=[list(range(CORES))],
            ins=[w_in.opt()],
            outs=[w_out.opt()],
        )

        ident_h = const_pool.tile([P, P], F16)
        ident_b = const_pool.tile([P, P], BF16)
        ident_f = const_pool.tile([P, P], F32)
        masks.make_identity(nc, ident_f[:])
        nc.gpsimd.tensor_copy(ident_h[:], ident_f[:])
        nc.gpsimd.tensor_copy(ident_b[:], ident_f[:])
        ones_h = const_pool.tile([P, 1], F16)
        nc.gpsimd.memset(ones_h[:], 1.0)

        # ---- phase A: stream shard in; cast to fp16 hi+lo; accumulate stats
        mm1_ctx = tc.tile_pool(name="mm1ps", bufs=1, space="PSUM")
        mm1_ps = mm1_ctx.__enter__()
        cent_ps = [mm1_ps.tile([P, 512], F32, name=f"cent_ps{h}") for h in range(2)]
        cnt_ps = mm1_ps.tile([P, 1], F32)
        eh_tiles = []
        mm1_last = None
        for i in range(NCH):
            y_t = y_pool.tile([P, C], F32, tag="y")
            e_t = e_pool.tile([P, D], F32, tag="e")
            nc.sync.dma_start(out=y_t[:], in_=yt[i * P:(i + 1) * P, :])
            nc.sync.dma_start(out=e_t[:], in_=emb[i * P:(i + 1) * P, :])
            y_h = y_pool.tile([P, C], F16, tag="yh")
            e_h = persist.tile([P, D], F16, name=f"eh{i}")
            e_l = el_pool.tile([P, D], F16, tag="el")
            nc.vector.tensor_copy(y_h[:], y_t[:])
            nc.vector.tensor_copy(e_h[:], e_t[:])
            nc.vector.tensor_sub(e_l[:], e_t[:], e_h[:])
            eh_tiles.append(e_h)
            first, last = (i == 0), (i == NCH - 1)
            for h in range(2):
                nc.tensor.matmul(
                    cent_ps[h][:], lhsT=y_h[:],
                    rhs=e_h[:, h * 512:(h + 1) * 512],
                    start=first, stop=False,
                )
            for h in range(2):
                nc.tensor.matmul(
                    cent_ps[h][:], lhsT=y_h[:],
                    rhs=e_l[:, h * 512:(h + 1) * 512],
                    start=False, stop=last,
                )
            mm1_last = nc.tensor.matmul(
                cnt_ps[:], lhsT=y_h[:], rhs=ones_h[:],
                start=first, stop=last,
            )

        # ---- phase B: AllReduce the [C, D+1] stats across the 8 cores ----
        # All chain DMAs trigger from ACT (idle during the load) so they are
        # not queued behind the eh-writeback triggers on the sync queue.
        stat_sb = persist.tile([P, D + 1], F32)
        ar_in = dram_pool.tile([P, D + 1], F32)
        ar_out = dram_pool.tile([P, D + 1], F32)
        gcnt = persist.tile([P, 1], F32)
        gcent = [persist.tile([P, P], F32, name=f"gcent{j}") for j in range(DCH)]
        with tc.high_priority():
            nc.scalar.copy(stat_sb[:, 0:512], cent_ps[0][:])
            nc.scalar.copy(stat_sb[:, 512:1024], cent_ps[1][:])
            nc.scalar.copy(stat_sb[:, D:D + 1], cnt_ps[:])
            mm1_ctx.__exit__(None, None, None)
            nc.scalar.dma_start(out=ar_in[:], in_=stat_sb[:])
            nc.gpsimd.collective_compute(
                "AllReduce",
                mybir.AluOpType.add,
                replica_groups=[list(range(CORES))],
                ins=[ar_in.opt()],
                outs=[ar_out.opt()],
            )
            # counts column first (tiny) so the reciprocal chain starts
            # before the 512KB centroid payload finishes landing
            nc.scalar.dma_start(out=gcnt[:], in_=ar_out[:, D:D + 1])
            for j in range(DCH):
                nc.scalar.dma_start(out=gcent[j][:],
                                    in_=ar_out[:, j * P:(j + 1) * P])

        # ---- phase T: E^T via DMA transpose XBAR (no PE involved) ----
        # fp16 E -> DRAM scratch -> two large xbar transposes into SBUF.
        # The writeback is gated behind mm1's last matmul (~= load end) so it
        # does not steal HBM bandwidth from the input load.
        eh_dram = dram_pool.tile([NS, D], F16)
        # et layout: [dd, j, nn] -> row d = j*128+dd of E^T at [dd, j, nn]
        et = persist.tile([P, DCH, NS], F16)
        for i in range(NCH):
            wr = nc.sync.dma_start(
                out=eh_dram[i * P:(i + 1) * P, :], in_=eh_tiles[i][:])
            if i == 0:
                add_dep_helper(wr.ins, mm1_last.ins, sync=True,
                               reason="eh writeback after load/mm1")
        for h in range(2):
            nc.sync.dma_start_transpose(
                out=et[:, :, h * 1024:(h + 1) * 1024],
                in_=eh_dram[h * 1024:(h + 1) * 1024, :],
            )

        # ---- phase C2: cent2h = 2*cent/counts (fp16), sq_c, cent2h^T ----
        safe = smalls.tile([P, 1], F32)
        nc.vector.tensor_scalar_max(safe[:], gcnt[:], 1.0)
        r2 = smalls.tile([P, 1], F32)
        nc.vector.reciprocal(r2[:], safe[:])
        nc.vector.tensor_scalar_mul(r2[:], r2[:], 2.0)
        sq_tmp = persist.tile([P, D], F32)
        negsq = smalls.tile([P, 1], F32)
        cent2h = [persist.tile([P, P], F16, name=f"cent2h{j}") for j in range(DCH)]
        centT = [persist.tile([P, C], F16, name=f"centT{j}") for j in range(DCH)]
        with tc.tile_pool(name="trps", bufs=2, space="PSUM") as tr_ps:
            for j in range(DCH):
                # scale + cast in one ACT op: cent2h = f16(r2 * gcent)
                nc.scalar.activation(cent2h[j][:], gcent[j][:], AF.Copy,
                                     bias=0.0, scale=r2[:, 0:1])
                tp = tr_ps.tile([P, P], F16, tag="tr")
                nc.tensor.transpose(tp[:], cent2h[j][:], ident_h[:])
                nc.vector.tensor_copy(centT[j][:], tp[:])
                # negsq contribution off the critical path (only exp needs it)
                nc.scalar.square(sq_tmp[:, j * P:(j + 1) * P], cent2h[j][:])
        nc.vector.reduce_sum(out=negsq[:], in_=sq_tmp[:], axis=AX.X)
        nc.vector.tensor_scalar_mul(negsq[:], negsq[:], -0.25)

        # ---- phase D/E: cross2 = cent2 @ E^T -> exp -> transpose -> softmax
        with (
            tc.tile_pool(name="crossps", bufs=1, space="PSUM") as cross_pool,
            tc.tile_pool(name="tr2ps", bufs=4, space="PSUM") as tr2_ps,
            tc.tile_pool(name="exps", bufs=NB) as exp_pool,
            tc.tile_pool(name="outtiles", bufs=4) as out_pool,
            tc.tile_pool(name="sums", bufs=8) as sum_pool,
        ):
            crs = [cross_pool.tile([P, 512], F32, name=f"cr{b}") for b in range(NB)]
            for j in range(DCH):
                for b in range(NB):
                    nc.tensor.matmul(
                        crs[b][:],
                        lhsT=centT[j][:],
                        rhs=et[:, j, b * 512:(b + 1) * 512],
                        start=(j == 0), stop=(j == DCH - 1),
                    )
            for b in range(NB):
                # exp(cross2 - sq_c) with per-partition bias; [C, 512] layout
                ex = exp_pool.tile([P, 512], BF16, tag="exp")
                nc.scalar.activation(ex[:], crs[b][:], AF.Exp, bias=negsq[:, 0:1],
                                     scale=1.0)
                # back to [n, C] orientation in 128-col strips, then normalize
                for tt in range(4):
                    t = b * 4 + tt
                    tp2 = tr2_ps.tile([P, P], BF16, tag="tr2")
                    nc.tensor.transpose(tp2[:], ex[:, tt * P:(tt + 1) * P],
                                        ident_b[:])
                    s = sum_pool.tile([P, 1], F32, tag="s")
                    nc.vector.reduce_sum(out=s[:], in_=tp2[:], axis=AX.X)
                    rs = sum_pool.tile([P, 1], F32, tag="rs")
                    nc.vector.reciprocal(rs[:], s[:])
                    ot = out_pool.tile([P, C], F32, tag="ot")
                    nc.scalar.activation(ot[:], tp2[:], AF.Copy, bias=0.0,
                                         scale=rs[:, 0:1])
                    nc.sync.dma_start(out=out[t * P:(t + 1) * P, :], in_=ot[:])


def build_module():
    nc = bacc.Bacc("TRN2", target_bir_lowering=False, debug=False,
                   num_devices=CORES)
    emb = nc.dram_tensor("embeddings", [NS, D], F32, kind="ExternalInput").ap()
    yt = nc.dram_tensor("y_true", [NS, C], F32, kind="ExternalInput").ap()
    out = nc.dram_tensor("out", [NS, C], F32, kind="ExternalOutput").ap()
    with tile.TileContext(nc) as tc:
        _build_kernel(tc, emb, yt, out)
    nc.compile()
    return nc


_NC_CACHE = {}


def _get_nc():
    if "nc" not in _NC_CACHE:
        _NC_CACHE["nc"] = build_module()
    return _NC_CACHE["nc"]


def run(embeddings: np.ndarray, y_true: np.ndarray, **spmd_kwargs):
    embeddings = np.ascontiguousarray(embeddings, dtype=np.float32)
    y_true = np.ascontiguousarray(y_true, dtype=np.float32)
    assert embeddings.shape == (N, D) and y_true.shape == (N, C)

    nc = _get_nc()
    in_maps = [
        {
            "embeddings": embeddings[k * NS:(k + 1) * NS],
            "y_true": y_true[k * NS:(k + 1) * NS],
        }
        for k in range(CORES)
    ]
    res = run_bass_kernel_spmd(nc, in_maps, core_ids=list(range(CORES)),
                               **spmd_kwargs)
    out = np.concatenate([res.results[k]["out"] for k in range(CORES)], axis=0)
    return out, res


def kernel(embeddings: np.ndarray, y_true: np.ndarray) -> np.ndarray:
    out, _ = run(embeddings, y_true)
    return out


# revision 11
# speedup vs baseline: 1.0432x; 1.0432x over previous
"""Centroid-similarity (ProtoNet softmax) kernel for 8 trn2 NeuronCores.

Math (per reference):
    counts   = sum_n y[n, c]
    cent     = (y^T @ E) / max(counts, 1)          # divide_no_nan
    out      = softmax(-(|e|^2 + |c|^2 - 2 e.c), axis=C)
softmax is invariant to per-row constants, so |e|^2 drops out:
    out      = softmax(2*cross - sq_c), cross = E @ cent^T, sq_c = |cent|^2

Distribution: data-parallel over N. Each core gets an N/8 = 2048-row shard,
computes partial (y^T E | counts) stats with the tensor engine, AllReduces
the [C, D+1] stats, then computes its own 2048 x C block of logits+softmax.

Key design points (v3):
- All matmuls in fp16 (11 mantissa bits; end-to-end rel err ~1.5e-2 vs the
  2e-2 gate; fp16 512-col matmuls stream at ~214ns vs ~630ns for fp32r).
- E^T (mm2 contracts over D) comes from the DMA-transpose XBAR, not the PE:
  fp16 E is written back to a DRAM scratch after the load drains (gated so
  it doesn't steal HBM bandwidth from the input load), then 4 xbar ops
  (split across the two hwdge queues, sync+ACT) land E^T in SBUF during the
  AllReduce window. Zero PE time, zero ACT/DVE copy traffic.
- eh_dram lives in its own DRAM pool: DRAM hazard tracking is pool-granular
  and putting it in the collective's pool serializes the xbar after the
  AllReduce (measured +45us).
- No warm-up collective: each collective has a ~12us latency floor and they
  serialize on the CC stream, so a warm-up AllReduce costs more than the
  one-time barrier it hides.
- DMAs are batched (1 y load, 8 two-chunk E loads, 1 gcent read, 4 output
  stores) since each hwdge trigger costs ~600ns of queue time.
- Tail: cent scale-cast on DVE and squares on ACT (parallel), mm2 at full
  fp16 rate, exp in bf16, PE transposes back to [n, C], per-strip
  normalization with the scale ops split ACT/gpsimd so no engine serializes
  the epilogue.
"""

import numpy as np

import concourse.bacc as bacc
import concourse.bass as bass
import concourse.mybir as mybir
import concourse.tile as tile
from concourse import masks
from concourse.bass_utils import run_bass_kernel_spmd
from concourse.tile import add_dep_helper

N, C, D = 16384, 128, 1024
CORES = 8
NS = N // CORES          # 2048 rows per core
P = 128                  # partition dim
NCH = NS // P            # 16 n-chunks per core
NPR = NCH // 2           # 8 two-chunk load pairs
DCH = D // P             # 8 d-chunks
NB = NS // 512           # 4 moving-dim blocks for matmul #2

F32 = mybir.dt.float32
F16 = mybir.dt.float16
BF16 = mybir.dt.bfloat16

AF = mybir.ActivationFunctionType
AX = mybir.AxisListType


def _build_kernel(tc: tile.TileContext, emb: bass.AP, yt: bass.AP, out: bass.AP):
    nc = tc.nc

    with (
        tc.tile_pool(name="const", bufs=1) as const_pool,
        tc.tile_pool(name="persist", bufs=1) as persist,
        tc.tile_pool(name="echunks", bufs=3) as e_pool,
        tc.tile_pool(name="dram", bufs=1, space="DRAM") as dram_pool,
        tc.tile_pool(name="drameh", bufs=1, space="DRAM") as drameh_pool,
        tc.tile_pool(name="smalls", bufs=1) as smalls,
    ):
        ident_h = const_pool.tile([P, P], F16)
        ident_b = const_pool.tile([P, P], BF16)
        ident_f = const_pool.tile([P, P], F32)
        masks.make_identity(nc, ident_f[:])
        nc.gpsimd.tensor_copy(ident_h[:], ident_f[:])
        nc.gpsimd.tensor_copy(ident_b[:], ident_f[:])
        ones_h = const_pool.tile([P, 1], F16)
        nc.gpsimd.memset(ones_h[:], 1.0)

        # ---- phase A: stream shard in; cast fp16; accumulate y^T E ----
        mm1_ctx = tc.tile_pool(name="mm1ps", bufs=1, space="PSUM")
        mm1_ps = mm1_ctx.__enter__()
        cent_ps = [mm1_ps.tile([P, 512], F32, name=f"cent_ps{h}") for h in range(2)]
        cnt_ps = mm1_ps.tile([P, 1], F32)

        y_all = persist.tile([P, NCH, C], F32)
        nc.sync.dma_start(
            out=y_all[:], in_=yt.rearrange("(i p) c -> p i c", p=P))
        y_h = persist.tile([P, NCH, C], F16)

        eh_tiles = []
        mm1_last = None
        for r in range(NPR):
            e_t = e_pool.tile([P, 2, D], F32, tag="e")
            nc.sync.dma_start(
                out=e_t[:],
                in_=emb[r * 2 * P:(r + 1) * 2 * P, :].rearrange(
                    "(k p) d -> p k d", p=P))
            e_h = persist.tile([P, 2, D], F16, name=f"eh{r}")
            nc.vector.tensor_copy(e_h[:], e_t[:])
            if r == 0:
                # one cast for the whole y shard (tiny vs the e casts)
                nc.vector.tensor_copy(y_h[:], y_all[:])
            eh_tiles.append(e_h)
            for k in range(2):
                i = 2 * r + k
                first, last = (i == 0), (i == NCH - 1)
                for h in range(2):
                    nc.tensor.matmul(
                        cent_ps[h][:], lhsT=y_h[:, i, :],
                        rhs=e_h[:, k, h * 512:(h + 1) * 512],
                        start=first, stop=last,
                    )
                mm1_last = nc.tensor.matmul(
                    cnt_ps[:], lhsT=y_h[:, i, :], rhs=ones_h[:],
                    start=first, stop=last,
                )

        # ---- phase B: AllReduce the [C, D+1] stats across the 8 cores ----
        # chain DMAs trigger from ACT so they never queue behind the
        # eh-writeback / xbar work on the sync queue
        stat_sb = persist.tile([P, D + 1], F32)
        ar_in = dram_pool.tile([P, D + 1], F32)
        ar_out = dram_pool.tile([P, D + 1], F32)
        gcnt = smalls.tile([P, 1], F32)
        gcent = persist.tile([P, D], F32)
        with tc.high_priority():
            nc.scalar.copy(stat_sb[:, 0:512], cent_ps[0][:])
            nc.scalar.copy(stat_sb[:, 512:1024], cent_ps[1][:])
            nc.scalar.copy(stat_sb[:, D:D + 1], cnt_ps[:])
            mm1_ctx.__exit__(None, None, None)
            nc.scalar.dma_start(out=ar_in[:], in_=stat_sb[:])
            nc.gpsimd.collective_compute(
                "AllReduce",
                mybir.AluOpType.add,
                replica_groups=[list(range(CORES))],
                ins=[ar_in.opt()],
                outs=[ar_out.opt()],
            )
            # counts column first (tiny) so the reciprocal chain starts
            # before the 512KB centroid payload finishes landing
            nc.scalar.dma_start(out=gcnt[:], in_=ar_out[:, D:D + 1])
            nc.scalar.dma_start(out=gcent[:], in_=ar_out[:, 0:D])

        # ---- phase T: E^T via DMA-transpose XBAR (no PE involved) ----
        # writeback gated on mm1's last matmul (~= load end); 4 xbar ops,
        # 2 per hwdge queue, each transposing a 512-row slab of E
        eh_dram = drameh_pool.tile([NS, D], F16)
        # et layout: [dd, j, nn] -> E^T row d = j*128+dd lives at [dd, j, nn]
        et = persist.tile([P, DCH, NS], F16)
        wr_insts = []
        for r in range(NPR):
            wr = nc.sync.dma_start(
                out=eh_dram[r * 2 * P:(r + 1) * 2 * P, :].rearrange(
                    "(k p) d -> p k d", p=P),
                in_=eh_tiles[r][:])
            if r == 0:
                add_dep_helper(wr.ins, mm1_last.ins, sync=True,
                               reason="eh writeback after load/mm1")
            wr_insts.append(wr)
        # all xbars on one queue: concurrent DMA_TRANSPOSE ucode on two hwdge
        # queues corrupts the shared descriptor scratch (measured: rows 5-7
        # mod 16 of the overlapping slabs land wrong)
        for h in range(4):
            eng = nc.sync
            xb = eng.dma_start_transpose(
                out=et[:, :, h * 512:(h + 1) * 512],
                in_=eh_dram[h * 512:(h + 1) * 512, :],
            )
            # DRAM write->read deps across DMA queues are not tracked at
            # completion granularity; wait for the two slab writebacks
            for r in (2 * h, 2 * h + 1):
                add_dep_helper(xb.ins, wr_insts[r].ins, sync=True,
                               reason="xbar reads eh_dram slab")

        # ---- phase C2: cent2h = f16(2*cent/counts), sq_c, cent2h^T ----
        safe = smalls.tile([P, 1], F32)
        nc.vector.tensor_scalar_max(safe[:], gcnt[:], 1.0)
        r2 = smalls.tile([P, 1], F32)
        nc.vector.reciprocal(r2[:], safe[:])
        nc.vector.tensor_scalar_mul(r2[:], r2[:], 2.0)
        sq_tmp = persist.tile([P, D], F32)
        negsq = smalls.tile([P, 1], F32)
        cent2h = persist.tile([P, D], F16)
        centT = [persist.tile([P, C], F16, name=f"centT{j}") for j in range(DCH)]
        with tc.tile_pool(name="trps", bufs=2, space="PSUM") as tr_ps:
            for j in range(DCH):
                sl = slice(j * P, (j + 1) * P)
                # scale + cast on DVE; squares on ACT (parallel chains)
                nc.vector.tensor_scalar_mul(cent2h[:, sl], gcent[:, sl],
                                            r2[:, 0:1])
                tp = tr_ps.tile([P, P], F16, tag="tr")
                nc.tensor.transpose(tp[:], cent2h[:, sl], ident_h[:])
                nc.vector.tensor_copy(centT[j][:], tp[:])
                nc.scalar.square(sq_tmp[:, sl], cent2h[:, sl])
        nc.vector.reduce_sum(out=negsq[:], in_=sq_tmp[:], axis=AX.X)
        nc.vector.tensor_scalar_mul(negsq[:], negsq[:], -0.25)

        # ---- phase D/E: cross2 = cent2 @ E^T -> exp -> transpose -> softmax
        with (
            tc.tile_pool(name="crossps", bufs=1, space="PSUM") as cross_pool,
            tc.tile_pool(name="tr2ps", bufs=4, space="PSUM") as tr2_ps,
            tc.tile_pool(name="exps", bufs=NB) as exp_pool,
            tc.tile_pool(name="outtiles", bufs=2) as out_pool,
            tc.tile_pool(name="sums", bufs=8) as sum_pool,
        ):
            crs = [cross_pool.tile([P, 512], F32, name=f"cr{b}") for b in range(NB)]
            for j in range(DCH):
                for b in range(NB):
                    nc.tensor.matmul(
                        crs[b][:],
                        lhsT=centT[j][:],
                        rhs=et[:, j, b * 512:(b + 1) * 512],
                        start=(j == 0), stop=(j == DCH - 1),
                    )
            for b in range(NB):
                # exp(cross2 - sq_c) with per-partition bias; [C, 512] layout
                ex = exp_pool.tile([P, 512], BF16, tag="exp")
                nc.scalar.activation(ex[:], crs[b][:], AF.Exp, bias=negsq[:, 0:1],
                                     scale=1.0)
                ot = out_pool.tile([P, 4, C], F32, tag="ot")
                for tt in range(4):
                    tp2 = tr2_ps.tile([P, P], BF16, tag="tr2")
                    nc.tensor.transpose(tp2[:], ex[:, tt * P:(tt + 1) * P],
                                        ident_b[:])
                    s = sum_pool.tile([P, 1], F32, tag="s")
                    nc.vector.reduce_sum(out=s[:], in_=tp2[:], axis=AX.X)
                    rs = sum_pool.tile([P, 1], F32, tag="rs")
                    nc.vector.reciprocal(rs[:], s[:])
                    # normalization scales alternate ACT / DVE so neither
                    # serializes the 16-strip epilogue (gpsimd can't read PSUM)
                    if tt % 2 == 0:
                        nc.scalar.activation(ot[:, tt, :], tp2[:], AF.Copy,
                                             bias=0.0, scale=rs[:, 0:1])
                    else:
                        nc.vector.tensor_scalar_mul(ot[:, tt, :], tp2[:],
                                                    rs[:, 0:1])
                nc.sync.dma_start(
                    out=out[b * 512:(b + 1) * 512, :].rearrange(
                        "(t p) c -> p t c", p=P),
                    in_=ot[:])


def build_module():
    nc = bacc.Bacc("TRN2", target_bir_lowering=False, debug=False,
                   num_devices=CORES)
    emb = nc.dram_tensor("embeddings", [NS, D], F32, kind="ExternalInput").ap()
    yt = nc.dram_tensor("y_true", [NS, C], F32, kind="ExternalInput").ap()
    out = nc.dram_tensor("out", [NS, C], F32, kind="ExternalOutput").ap()
    with tile.TileContext(nc) as tc:
        _build_kernel(tc, emb, yt, out)
    nc.compile()
    return nc


_NC_CACHE = {}


def _get_nc():
    if "nc" not in _NC_CACHE:
        _NC_CACHE["nc"] = build_module()
    return _NC_CACHE["nc"]


def run(embeddings: np.ndarray, y_true: np.ndarray, **spmd_kwargs):
    embeddings = np.ascontiguousarray(embeddings, dtype=np.float32)
    y_true = np.ascontiguousarray(y_true, dtype=np.float32)
    assert embeddings.shape == (N, D) and y_true.shape == (N, C)

    nc = _get_nc()
    in_maps = [
        {
            "embeddings": embeddings[k * NS:(k + 1) * NS],
            "y_true": y_true[k * NS:(k + 1) * NS],
        }
        for k in range(CORES)
    ]
    res = run_bass_kernel_spmd(nc, in_maps, core_ids=list(range(CORES)),
                               **spmd_kwargs)
    out = np.concatenate([res.results[k]["out"] for k in range(CORES)], axis=0)
    return out, res


def kernel(embeddings: np.ndarray, y_true: np.ndarray) -> np.ndarray:
    out, _ = run(embeddings, y_true)
    return out


# revision 12
# speedup vs baseline: 1.2138x; 1.1636x over previous
"""Centroid-similarity (ProtoNet softmax) kernel for 8 trn2 NeuronCores.

Math (per reference):
    counts   = sum_n y[n, c]
    cent     = (y^T @ E) / max(counts, 1)          # divide_no_nan
    out      = softmax(-(|e|^2 + |c|^2 - 2 e.c), axis=C)
softmax is invariant to per-row constants, so |e|^2 drops out:
    out      = softmax(2*cross - sq_c), cross = E @ cent^T, sq_c = |cent|^2

Distribution: data-parallel over N. Each core gets an N/8 = 2048-row shard,
computes partial (y^T E | counts) stats with the tensor engine, AllReduces
the [C, D+1] stats, then computes its own 2048 x C block of logits+softmax.

Key design points (v4):
- Everything 16-bit on the PE: fp16 operands (11 mantissa bits, like the
  fp32r path but ~2-3x faster: 512-col matmuls stream at ~214ns) with fp32
  PSUM accumulation. End-to-end rel err ~1.5e-2 vs the 2e-2 gate.
- Batched DMAs (1 y load, 8 two-chunk E loads, 4 output stores): each hwdge
  trigger costs ~600ns of queue time; batching got the input load to
  ~370GB/s, finishing (and triggering the AllReduce chain) by ~50us.
- The stats AllReduce is split in two ([C,513] with counts first, then
  [C,512]) from separate DRAM pools (DRAM hazards are pool-granular).
  Consecutive CC ops pipeline their setup behind the предыдущий transfer,
  so the split lets the first half of the centroid chain + mm2 overlap the
  second half's wire time. Collectives can't start before the one-time CC
  barrier (~60-70us) no matter when triggered; a warm-up collective doesn't
  help (each CC op has a ~12us latency floor and they serialize).
- E^T for mm2 is built by 128 fp16 PE transposes during the AllReduce
  window (PE is idle then). The DMA-transpose XBAR would be free, but the
  tile framework serializes DMA transposes against collectives, which puts
  them after the AllReduce - worse. PSUM->SBUF copies alternate ACT/DVE.
- AllReduce-adjacent DMAs trigger from the sync queue so the blocked gcnt/
  gcent reads don't head-of-line-block the ACT/DVE transpose copies.
- Epilogue: exp -> bf16, PE transposes back to [n, C], DVE row-sums +
  reciprocals, normalization scales alternating ACT/DVE, one output DMA
  per 512-row block.
"""

import numpy as np

import concourse.bacc as bacc
import concourse.bass as bass
import concourse.mybir as mybir
import concourse.tile as tile
from concourse import masks
from concourse.bass_utils import run_bass_kernel_spmd
from concourse.tile import add_dep_helper

N, C, D = 16384, 128, 1024
CORES = 8
NS = N // CORES          # 2048 rows per core
P = 128                  # partition dim
NCH = NS // P            # 16 n-chunks per core
NPR = NCH // 2           # 8 two-chunk load pairs
DCH = D // P             # 8 d-chunks
NB = NS // 512           # 4 moving-dim blocks for matmul #2

F32 = mybir.dt.float32
F16 = mybir.dt.float16
BF16 = mybir.dt.bfloat16

AF = mybir.ActivationFunctionType
AX = mybir.AxisListType

AR_SPLIT = True          # two pipelined stats AllReduces instead of one


def _build_kernel(tc: tile.TileContext, emb: bass.AP, yt: bass.AP, out: bass.AP):
    nc = tc.nc

    with (
        tc.tile_pool(name="const", bufs=1) as const_pool,
        tc.tile_pool(name="persist", bufs=1) as persist,
        tc.tile_pool(name="echunks", bufs=3) as e_pool,
        tc.tile_pool(name="drama", bufs=1, space="DRAM") as dram_a,
        tc.tile_pool(name="dramb", bufs=1, space="DRAM") as dram_b,
        tc.tile_pool(name="smalls", bufs=1) as smalls,
    ):
        ident_h = const_pool.tile([P, P], F16)
        ident_b = const_pool.tile([P, P], BF16)
        ident_f = const_pool.tile([P, P], F32)
        masks.make_identity(nc, ident_f[:])
        nc.gpsimd.tensor_copy(ident_h[:], ident_f[:])
        nc.gpsimd.tensor_copy(ident_b[:], ident_f[:])
        ones_h = const_pool.tile([P, 1], F16)
        nc.gpsimd.memset(ones_h[:], 1.0)

        # ---- phase A: stream shard in; cast fp16; accumulate y^T E ----
        mm1_ctx = tc.tile_pool(name="mm1ps", bufs=1, space="PSUM")
        mm1_ps = mm1_ctx.__enter__()
        cent_ps = [mm1_ps.tile([P, 512], F32, name=f"cent_ps{h}") for h in range(2)]
        cnt_ps = mm1_ps.tile([P, 1], F32)

        y_all = persist.tile([P, NCH, C], F32)
        nc.sync.dma_start(
            out=y_all[:], in_=yt.rearrange("(i p) c -> p i c", p=P))
        y_h = persist.tile([P, NCH, C], F16)

        eh_tiles = []
        mm1_last = None
        for r in range(NPR):
            e_t = e_pool.tile([P, 2, D], F32, tag="e")
            nc.sync.dma_start(
                out=e_t[:],
                in_=emb[r * 2 * P:(r + 1) * 2 * P, :].rearrange(
                    "(k p) d -> p k d", p=P))
            e_h = persist.tile([P, 2, D], F16, name=f"eh{r}")
            nc.vector.tensor_copy(e_h[:], e_t[:])
            if r == 0:
                # one cast for the whole y shard (tiny vs the e casts)
                nc.vector.tensor_copy(y_h[:], y_all[:])
            eh_tiles.append(e_h)
            for k in range(2):
                i = 2 * r + k
                first, last = (i == 0), (i == NCH - 1)
                for h in range(2):
                    nc.tensor.matmul(
                        cent_ps[h][:], lhsT=y_h[:, i, :],
                        rhs=e_h[:, k, h * 512:(h + 1) * 512],
                        start=first, stop=last,
                    )
                mm1_last = nc.tensor.matmul(
                    cnt_ps[:], lhsT=y_h[:, i, :], rhs=ones_h[:],
                    start=first, stop=last,
                )

        # ---- phase B: AllReduce [C, D+1] stats (optionally split in two,
        # counts+lower D first) across the 8 cores. DMAs trigger from sync:
        # the post-AR reads block there without stalling ACT/DVE.
        gcnt = smalls.tile([P, 1], F32)
        gcent = persist.tile([P, D], F32)
        groups = [list(range(CORES))]
        with tc.high_priority():
            if AR_SPLIT:
                stat_a = persist.tile([P, 513], F32)
                stat_b = persist.tile([P, 512], F32)
                nc.scalar.copy(stat_a[:, 0:512], cent_ps[0][:])
                nc.scalar.copy(stat_a[:, 512:513], cnt_ps[:])
                nc.scalar.copy(stat_b[:], cent_ps[1][:])
                mm1_ctx.__exit__(None, None, None)
                ar_in_a = dram_a.tile([P, 513], F32)
                ar_out_a = dram_a.tile([P, 513], F32)
                ar_in_b = dram_b.tile([P, 512], F32)
                ar_out_b = dram_b.tile([P, 512], F32)
                nc.sync.dma_start(out=ar_in_a[:], in_=stat_a[:])
                nc.sync.dma_start(out=ar_in_b[:], in_=stat_b[:])
                nc.gpsimd.collective_compute(
                    "AllReduce", mybir.AluOpType.add, replica_groups=groups,
                    ins=[ar_in_a.opt()], outs=[ar_out_a.opt()])
                nc.gpsimd.collective_compute(
                    "AllReduce", mybir.AluOpType.add, replica_groups=groups,
                    ins=[ar_in_b.opt()], outs=[ar_out_b.opt()])
                nc.sync.dma_start(out=gcnt[:], in_=ar_out_a[:, 512:513])
                nc.sync.dma_start(out=gcent[:, 0:512], in_=ar_out_a[:, 0:512])
                nc.sync.dma_start(out=gcent[:, 512:1024], in_=ar_out_b[:])
            else:
                stat_sb = persist.tile([P, D + 1], F32)
                nc.scalar.copy(stat_sb[:, 0:512], cent_ps[0][:])
                nc.scalar.copy(stat_sb[:, 512:1024], cent_ps[1][:])
                nc.scalar.copy(stat_sb[:, D:D + 1], cnt_ps[:])
                mm1_ctx.__exit__(None, None, None)
                ar_in = dram_a.tile([P, D + 1], F32)
                ar_out = dram_a.tile([P, D + 1], F32)
                nc.sync.dma_start(out=ar_in[:], in_=stat_sb[:])
                nc.gpsimd.collective_compute(
                    "AllReduce", mybir.AluOpType.add, replica_groups=groups,
                    ins=[ar_in.opt()], outs=[ar_out.opt()])
                nc.sync.dma_start(out=gcnt[:], in_=ar_out[:, D:D + 1])
                nc.sync.dma_start(out=gcent[:], in_=ar_out[:, 0:D])

        # ---- phase T: E^T via PE transposes during the AllReduce window --
        # et layout: [dd, j, nn] -> E^T row d = j*128+dd lives at [dd, j, nn]
        et = persist.tile([P, DCH, NS], F16)
        with tc.tile_pool(name="trT", bufs=8, space="PSUM") as trT_ps:
            kk = 0
            for r in range(NPR):
                for k in range(2):
                    i = 2 * r + k
                    for j in range(DCH):
                        tp = trT_ps.tile([P, P], F16, tag="trT")
                        tr_inst = nc.tensor.transpose(
                            tp[:], eh_tiles[r][:, k, j * P:(j + 1) * P],
                            ident_h[:])
                        # ordering-only edge: keep the PE transposes behind
                        # mm1 in the queue so mm1 never waits
                        add_dep_helper(tr_inst.ins, mm1_last.ins, sync=False,
                                       reason="transposes after mm1")
                        dst = et[:, j, i * P:(i + 1) * P]
                        if kk % 2 == 0:
                            nc.scalar.copy(dst, tp[:])
                        else:
                            nc.vector.tensor_copy(dst, tp[:])
                        kk += 1

        # ---- phase C2: cent2h = f16(2*cent/counts), sq_c, cent2h^T ----
        safe = smalls.tile([P, 1], F32)
        nc.vector.tensor_scalar_max(safe[:], gcnt[:], 1.0)
        r2 = smalls.tile([P, 1], F32)
        nc.vector.reciprocal(r2[:], safe[:])
        nc.vector.tensor_scalar_mul(r2[:], r2[:], 2.0)
        sq_tmp = persist.tile([P, D], F32)
        negsq = smalls.tile([P, 1], F32)
        cent2h = persist.tile([P, D], F16)
        centT = [persist.tile([P, C], F16, name=f"centT{j}") for j in range(DCH)]
        with tc.tile_pool(name="trps", bufs=2, space="PSUM") as tr_ps:
            for j in range(DCH):
                sl = slice(j * P, (j + 1) * P)
                # scale + cast on DVE; squares on ACT (parallel chains)
                nc.vector.tensor_scalar_mul(cent2h[:, sl], gcent[:, sl],
                                            r2[:, 0:1])
                tp = tr_ps.tile([P, P], F16, tag="tr")
                nc.tensor.transpose(tp[:], cent2h[:, sl], ident_h[:])
                nc.vector.tensor_copy(centT[j][:], tp[:])
                nc.scalar.square(sq_tmp[:, sl], cent2h[:, sl])
        nc.vector.reduce_sum(out=negsq[:], in_=sq_tmp[:], axis=AX.X)
        nc.vector.tensor_scalar_mul(negsq[:], negsq[:], -0.25)

        # ---- phase D/E: cross2 = cent2 @ E^T -> exp -> transpose -> softmax
        with (
            tc.tile_pool(name="crossps", bufs=1, space="PSUM") as cross_pool,
            tc.tile_pool(name="tr2ps", bufs=4, space="PSUM") as tr2_ps,
            tc.tile_pool(name="exps", bufs=NB) as exp_pool,
            tc.tile_pool(name="outtiles", bufs=2) as out_pool,
            tc.tile_pool(name="sums", bufs=8) as sum_pool,
        ):
            crs = [cross_pool.tile([P, 512], F32, name=f"cr{b}") for b in range(NB)]
            for j in range(DCH):
                for b in range(NB):
                    nc.tensor.matmul(
                        crs[b][:],
                        lhsT=centT[j][:],
                        rhs=et[:, j, b * 512:(b + 1) * 512],
                        start=(j == 0), stop=(j == DCH - 1),
                    )
            for b in range(NB):
                # exp(cross2 - sq_c) with per-partition bias; [C, 512] layout
                ex = exp_pool.tile([P, 512], BF16, tag="exp")
                nc.scalar.activation(ex[:], crs[b][:], AF.Exp, bias=negsq[:, 0:1],
                                     scale=1.0)
                ot = out_pool.tile([P, 4, C], F32, tag="ot")
                for tt in range(4):
                    tp2 = tr2_ps.tile([P, P], BF16, tag="tr2")
                    nc.tensor.transpose(tp2[:], ex[:, tt * P:(tt + 1) * P],
                                        ident_b[:])
                    s = sum_pool.tile([P, 1], F32, tag="s")
                    nc.vector.reduce_sum(out=s[:], in_=tp2[:], axis=AX.X)
                    rs = sum_pool.tile([P, 1], F32, tag="rs")
                    nc.vector.reciprocal(rs[:], s[:])
                    # normalization scales alternate ACT / DVE so neither
                    # serializes the 16-strip epilogue
                    if tt % 2 == 0:
                        nc.scalar.activation(ot[:, tt, :], tp2[:], AF.Copy,
                                             bias=0.0, scale=rs[:, 0:1])
                    else:
                        nc.vector.tensor_scalar_mul(ot[:, tt, :], tp2[:],
                                                    rs[:, 0:1])
                nc.sync.dma_start(
                    out=out[b * 512:(b + 1) * 512, :].rearrange(
                        "(t p) c -> p t c", p=P),
                    in_=ot[:])


def build_module():
    nc = bacc.Bacc("TRN2", target_bir_lowering=False, debug=False,
                   num_devices=CORES)
    emb = nc.dram_tensor("embeddings", [NS, D], F32, kind="ExternalInput").ap()
    yt = nc.dram_tensor("y_true", [NS, C], F32, kind="ExternalInput").ap()
    out = nc.dram_tensor("out", [NS, C], F32, kind="ExternalOutput").ap()
    with tile.TileContext(nc) as tc:
        _build_kernel(tc, emb, yt, out)
    nc.compile()
    return nc


_NC_CACHE = {}


def _get_nc():
    if "nc" not in _NC_CACHE:
        _NC_CACHE["nc"] = build_module()
    return _NC_CACHE["nc"]


def run(embeddings: np.ndarray, y_true: np.ndarray, **spmd_kwargs):
    embeddings = np.ascontiguousarray(embeddings, dtype=np.float32)
    y_true = np.ascontiguousarray(y_true, dtype=np.float32)
    assert embeddings.shape == (N, D) and y_true.shape == (N, C)

    nc = _get_nc()
    in_maps = [
        {
            "embeddings": embeddings[k * NS:(k + 1) * NS],
            "y_true": y_true[k * NS:(k + 1) * NS],
        }
        for k in range(CORES)
    ]
    res = run_bass_kernel_spmd(nc, in_maps, core_ids=list(range(CORES)),
                               **spmd_kwargs)
    out = np.concatenate([res.results[k]["out"] for k in range(CORES)], axis=0)
    return out, res


def kernel(embeddings: np.ndarray, y_true: np.ndarray) -> np.ndarray:
    out, _ = run(embeddings, y_true)
    return out
